# revision 4
# baseline (speedup 1.0000x reference)
"""Brute-force KNN retrieval (B=512 queries, N=500000 candidates, D=128, top-K)
on 8 Trainium2 NeuronCores.

Strategy (sharding_hint): candidates sharded along N across the 8 cores,
queries replicated. Per core, per 2048-wide PSUM chunk:
  - PE computes bf16 scores (fp32 PSUM).
  - The chunk is drained to a contiguous fp16(score+128) SBUF tile. GPSIMD
    cannot touch PSUM and only ACT/DVE can, so the drain is split: ACT
    (activation Copy, most chunks) and DVE (tensor_scalar_add, ~1/28 of
    chunks) to balance engine load.
  - DVE tensor_max in fp16 (2x perf mode) folds halves twice:
    2048 -> 1024 -> 512. Element j of the result is the max of the quad
    {j, j+512, j+1024, j+1536} of the chunk.
  - Pool copies the 512 quad-winners into the HIGH int16 lanes of a
    persistent packed fp32 tile whose LOW lanes hold iota (0..511); for
    positive fp16 the fp32 word orders by (fp16 value, quad id).
  - DVE max8 emits the chunk's top-8 (value, quad id) pairs in one pass.
Keep-top-8-quads-per-2048-chunk covers the true top-100-of-500000 exactly
as well as top-8 singles (a quad ranks >= its members), so P[miss] ~ 2e-8.
The host expands surviving quads (4 members each), rescores the top ~256
quads per row exactly in fp32, and emits the exact global top-K
(ties -> lower index, like lax.top_k).
"""

import sys

for _p in ("/opt/trn_rl_repo",):
    if _p not in sys.path:
        sys.path.insert(0, _p)

import numpy as np

B, N, D = 512, 500000, 128
K = 100
N_CORES = 8
SHARD = N // N_CORES          # 62500 candidates per core
PCHUNK = 2048                 # PSUM tile width (4 banks)
QUAD = PCHUNK // 4            # 512 quad-winners per chunk
NCHUNK = -(-SHARD // PCHUNK)  # 31
PADN = PCHUNK * NCHUNK        # 63488 (padded shard width)
NSUB = PCHUNK // 512          # 4 matmuls per PSUM tile
MTILES = B // 128             # 4 query tiles
SURV = NCHUNK * 8             # 248 surviving quads per (row, core)
ST_BUFS = 4                   # fp16 drain tiles
PK_BUFS = 4                   # persistent packed tiles (iota-carrying)
RESCORE = 256                 # host rescores this many quads per row
BIAS = 128.0                  # score bias -> positive range for bit-ordering

_NC_CACHE = {}


def _is_dve_drain(t):
    # ~1/28 of chunk-tiles drain via DVE to relieve the ACT engine
    return t % 28 == 13


def _build_nc():
    import concourse.bacc as bacc
    import concourse.tile as tile
    import concourse.mybir as mybir

    f32 = mybir.dt.float32
    f16 = mybir.dt.float16
    u16 = mybir.dt.uint16
    bf16 = mybir.dt.bfloat16

    nc = bacc.Bacc(
        "TRN2", target_bir_lowering=False, debug=False, num_devices=N_CORES
    )
    qT = nc.dram_tensor("qT", [D, B], bf16, kind="ExternalInput")
    cT = nc.dram_tensor("cT", [D, PADN], bf16, kind="ExternalInput")
    packed = nc.dram_tensor("packed", [B, SURV], f32, kind="ExternalOutput")

    with tile.TileContext(nc) as tc:
        with (
            tc.tile_pool(name="q", bufs=1) as qp,
            tc.tile_pool(name="c", bufs=4) as cp,
            tc.tile_pool(name="ps", bufs=2, space="PSUM") as pp,
            tc.tile_pool(name="st", bufs=ST_BUFS) as stp,
            tc.tile_pool(name="r1", bufs=4) as r1p,
            tc.tile_pool(name="r2", bufs=4) as r2p,
            tc.tile_pool(name="pk", bufs=1) as pkp,
            tc.tile_pool(name="acc", bufs=1) as op,
        ):
            qt = qp.tile([128, B], bf16)
            nc.sync.dma_start(qt[:], qT.ap())

            pacc = [
                op.tile([128, SURV], f32, name=f"pacc{m}", tag=f"p{m}")
                for m in range(MTILES)
            ]
            pk = [
                pkp.tile([128, QUAD], f32, name=f"pk{j}", tag=f"k{j}")
                for j in range(PK_BUFS)
            ]
            # one-time iota into the LOW int16 lane of each packed fp32
            for j in range(PK_BUFS):
                lo = pk[j][:].bitcast(u16).rearrange(
                    "p (n two) -> p n two", two=2
                )[:, :, 0]
                nc.gpsimd.iota(lo, pattern=[[1, QUAD]], base=0,
                               channel_multiplier=0)

            for c in range(NCHUNK):
                ct = cp.tile([128, PCHUNK], bf16, name=f"ct{c}", tag="ct")
                nc.sync.dma_start(ct[:], cT.ap()[:, c * PCHUNK:(c + 1) * PCHUNK])
                for m in range(MTILES):
                    t = c * MTILES + m
                    ps = pp.tile([128, PCHUNK], f32, name=f"ps{c}_{m}", tag="ps")
                    for s in range(NSUB):
                        nc.tensor.matmul(
                            ps[:, s * 512:(s + 1) * 512],
                            qt[:, m * 128:(m + 1) * 128],
                            ct[:, s * 512:(s + 1) * 512],
                            start=True,
                            stop=True,
                        )
                    # drain PSUM -> contiguous fp16(score+128)
                    st = stp.tile([128, PCHUNK], f16, name=f"st{c}_{m}", tag="st")
                    if _is_dve_drain(t):
                        nc.vector.tensor_scalar_add(st[:], ps[:], BIAS)
                    else:
                        nc.scalar.activation(
                            st[:], ps[:], mybir.ActivationFunctionType.Copy,
                            bias=BIAS, scale=1.0,
                        )
                    # DVE fp16 2x: fold halves twice (2048 -> 1024 -> 512)
                    r1 = r1p.tile([128, PCHUNK // 2], f16,
                                  name=f"r1_{c}_{m}", tag="r1")
                    nc.vector.tensor_max(
                        r1[:], st[:, :PCHUNK // 2], st[:, PCHUNK // 2:]
                    )
                    r2 = r2p.tile([128, QUAD], f16,
                                  name=f"r2_{c}_{m}", tag="r2")
                    nc.vector.tensor_max(
                        r2[:], r1[:, :QUAD], r1[:, QUAD:]
                    )
                    # Pool: quad-winners into HIGH lanes of packed tile
                    pj = pk[t % PK_BUFS]
                    hi = pj[:].bitcast(f16).rearrange(
                        "p (n two) -> p n two", two=2
                    )[:, :, 1]
                    nc.gpsimd.tensor_copy(hi, r2[:])
                    # DVE: top-8 packed (value, quad id) of the chunk
                    nc.vector.max(pacc[m][:, c * 8:(c + 1) * 8], pj[:])

            for m in range(MTILES):
                nc.sync.dma_start(packed.ap()[m * 128:(m + 1) * 128, :], pacc[m][:])

    nc.compile()
    return nc


def _get_nc():
    if "nc" not in _NC_CACHE:
        _NC_CACHE["nc"] = _build_nc()
    return _NC_CACHE["nc"]


def _make_in_maps(queries, candidates):
    import ml_dtypes

    bf = ml_dtypes.bfloat16
    q = np.asarray(queries, dtype=np.float32)
    cand = np.asarray(candidates, dtype=np.float32)
    qTh = np.ascontiguousarray(q.T.astype(bf))  # [D, B] bf16
    in_maps = []
    for i in range(N_CORES):
        cTi = np.zeros((D, PADN), dtype=bf)
        cTi[:, :SHARD] = cand[i * SHARD:(i + 1) * SHARD].T.astype(bf)
        in_maps.append({"qT": qTh, "cT": cTi})
    return in_maps


def _run_device(in_maps, trace=False):
    from concourse import bass_utils

    nc = _get_nc()
    return bass_utils.run_bass_kernel_spmd(
        nc, in_maps, core_ids=list(range(N_CORES)), trace=trace
    )


def _merge(results, queries, candidates, identifiers, num_candidates):
    K = int(num_candidates)
    q = np.asarray(queries, dtype=np.float32)
    cand = np.asarray(candidates, dtype=np.float32)
    chunk_base = np.repeat(np.arange(NCHUNK, dtype=np.int64) * PCHUNK, 8)  # [SURV]
    all_u = []
    all_g = []  # [B, SURV, 4] member global ids (-1 = invalid)
    for i in range(N_CORES):
        u = np.asarray(results[i]["packed"]).view(np.uint32)       # [B, SURV]
        j = (u & 0xFFFF).astype(np.int64)                          # quad id
        # quad members: j + 512*k within the chunk
        local = chunk_base[None, :, None] + j[:, :, None] \
            + QUAD * np.arange(4, dtype=np.int64)[None, None, :]   # [B,SURV,4]
        valid = local < SHARD
        u = np.where(valid.any(axis=2), u, 0)  # all-pad quads rank last
        g = np.where(valid, i * SHARD + local, -1)
        all_u.append(u)
        all_g.append(g)
    ucat = np.concatenate(all_u, axis=1)    # [B, 8*SURV]
    gcat = np.concatenate(all_g, axis=1)    # [B, 8*SURV, 4]
    # candidate set for exact rescoring: top RESCORE quads per row by rank
    nres = min(RESCORE, ucat.shape[1])
    part = np.argpartition(ucat, ucat.shape[1] - nres, axis=1)[:, -nres:]
    rows = np.arange(B)[:, None]
    gsel = gcat[rows, part].reshape(B, nres * 4)                   # [B, 4n]
    invalid = gsel < 0
    gsel_c = np.where(invalid, 0, gsel)
    # exact fp32 rescore: s[b, j] = q[b] . cand[gsel[b, j]]
    csel = cand[gsel_c]                                            # [B, 4n, D]
    vsel = np.einsum("bjd,bd->bj", csel, q, dtype=np.float32)
    vsel = np.where(invalid, -np.inf, vsel)
    # exact top-K, ties -> lower global index (matches lax.top_k)
    order = np.lexsort((gsel_c, -vsel), axis=-1)[:, :K]
    out_vals = np.take_along_axis(vsel, order, axis=1).astype(np.float32)
    out_gidx = np.take_along_axis(gsel_c, order, axis=1)
    ids = np.asarray(identifiers)
    out_ids = np.take(ids, out_gidx, axis=0)
    return out_vals, out_ids


def kernel(queries, candidates, identifiers, num_candidates):
    in_maps = _make_in_maps(queries, candidates)
    res = _run_device(in_maps, trace=False)
    return _merge(res.results, queries, candidates, identifiers, num_candidates)


# revision 5
# speedup vs baseline: 1.6745x; 1.6745x over previous
"""Brute-force KNN retrieval (B=512 queries, N=500000 candidates, D=128, top-K)
on 8 Trainium2 NeuronCores.

Strategy (sharding_hint): candidates sharded along N across the 8 cores,
queries replicated. Per core, per 8192-wide "superchunk" (4 PSUM chunks):
  - PE computes bf16 scores (fp32 PSUM), 2048 columns per PSUM tile.
  - Each PSUM chunk is drained to a quarter of a contiguous fp16(score+128)
    SBUF tile. Only ACT/DVE can read PSUM; the drain is split between ACT
    (activation Copy, ~8/9 of chunks) and DVE (tensor_scalar_add, ~1/9).
  - DVE tensor_max in fp16 (2x perf mode) folds the 8192-wide tile four
    times: 8192 -> 4096 -> 2048 -> 1024 -> 512. Element j of the result is
    the max of the 16-member group {j + 512k, k=0..15} of the superchunk.
  - DVE copies the 512 group-winners into the HIGH int16 lanes of a
    persistent packed fp32 tile whose LOW lanes hold iota (0..511); for
    positive fp16 the fp32 word orders by (fp16 value, group id).
  - DVE max8 emits the superchunk's top-8 (value, group id) in one pass.
Keep-top-8-groups-per-8192 covers the true top-100-of-500000 almost as
well as top-8 singles (a group ranks >= its members; a miss needs >=9 of
the row's top-100 inside one 8192-bin: P ~ 5e-5 per row).
The host expands surviving groups (16 members each), rescores the top
~256 groups per row exactly in fp32, and emits the exact global top-K
(ties -> lower index, like lax.top_k).
"""

import sys

for _p in ("/opt/trn_rl_repo",):
    if _p not in sys.path:
        sys.path.insert(0, _p)

import numpy as np

B, N, D = 512, 500000, 128
K = 100
N_CORES = 8
SHARD = N // N_CORES          # 62500 candidates per core
PCHUNK = 2048                 # PSUM tile width (4 banks)
SUPER = 4 * PCHUNK            # 8192-wide fold unit
GROUP = 512                   # group-winners per superchunk (16 members)
NSUPER = -(-SHARD // SUPER)   # 8
PADN = SUPER * NSUPER         # 65536 (padded shard width)
NCHUNK = PADN // PCHUNK       # 32
NSUB = PCHUNK // 512          # 4 matmuls per PSUM tile
MTILES = B // 128             # 4 query tiles
SURV = NSUPER * 8             # 64 surviving groups per (row, core)
ST_BUFS = 2                   # fp16 super drain tiles
PK_BUFS = 4                   # persistent packed tiles (iota-carrying)
RESCORE = 256                 # host rescores this many groups per row
BIAS = 128.0                  # score bias -> positive range for bit-ordering

_NC_CACHE = {}


def _is_dve_drain(t):
    # ~1/9 of chunk-tiles drain via DVE to relieve the ACT engine
    return t % 9 == 4


def _build_nc():
    import concourse.bacc as bacc
    import concourse.tile as tile
    import concourse.mybir as mybir

    f32 = mybir.dt.float32
    f16 = mybir.dt.float16
    u16 = mybir.dt.uint16
    bf16 = mybir.dt.bfloat16

    nc = bacc.Bacc(
        "TRN2", target_bir_lowering=False, debug=False, num_devices=N_CORES
    )
    qT = nc.dram_tensor("qT", [D, B], bf16, kind="ExternalInput")
    cT = nc.dram_tensor("cT", [D, PADN], bf16, kind="ExternalInput")
    packed = nc.dram_tensor("packed", [B, SURV], f32, kind="ExternalOutput")

    with tile.TileContext(nc) as tc:
        with (
            tc.tile_pool(name="q", bufs=1) as qp,
            tc.tile_pool(name="c", bufs=4) as cp,
            tc.tile_pool(name="ps", bufs=2, space="PSUM") as pp,
            tc.tile_pool(name="st", bufs=ST_BUFS) as stp,
            tc.tile_pool(name="a1", bufs=2) as a1p,
            tc.tile_pool(name="a2", bufs=2) as a2p,
            tc.tile_pool(name="a3", bufs=2) as a3p,
            tc.tile_pool(name="a4", bufs=2) as a4p,
            tc.tile_pool(name="pk", bufs=1) as pkp,
            tc.tile_pool(name="acc", bufs=1) as op,
        ):
            qt = qp.tile([128, B], bf16)
            nc.sync.dma_start(qt[:], qT.ap())

            pacc = [
                op.tile([128, SURV], f32, name=f"pacc{m}", tag=f"p{m}")
                for m in range(MTILES)
            ]
            pk = [
                pkp.tile([128, GROUP], f32, name=f"pk{j}", tag=f"k{j}")
                for j in range(PK_BUFS)
            ]
            # one-time iota into the LOW int16 lane of each packed fp32
            for j in range(PK_BUFS):
                lo = pk[j][:].bitcast(u16).rearrange(
                    "p (n two) -> p n two", two=2
                )[:, :, 0]
                nc.gpsimd.iota(lo, pattern=[[1, GROUP]], base=0,
                               channel_multiplier=0)

            tcount = 0
            for sidx in range(NSUPER):
                # DMA the 4 candidate chunks of this superchunk
                cts = []
                for q in range(4):
                    c = sidx * 4 + q
                    ct = cp.tile([128, PCHUNK], bf16, name=f"ct{c}", tag="ct")
                    nc.sync.dma_start(
                        ct[:], cT.ap()[:, c * PCHUNK:(c + 1) * PCHUNK]
                    )
                    cts.append(ct)
                for m in range(MTILES):
                    st = stp.tile([128, SUPER], f16,
                                  name=f"st{sidx}_{m}", tag="st")
                    for q in range(4):
                        ps = pp.tile([128, PCHUNK], f32,
                                     name=f"ps{sidx}_{m}_{q}", tag="ps")
                        for s in range(NSUB):
                            nc.tensor.matmul(
                                ps[:, s * 512:(s + 1) * 512],
                                qt[:, m * 128:(m + 1) * 128],
                                cts[q][:, s * 512:(s + 1) * 512],
                                start=True,
                                stop=True,
                            )
                        dst = st[:, q * PCHUNK:(q + 1) * PCHUNK]
                        if _is_dve_drain(tcount):
                            nc.vector.tensor_scalar_add(dst, ps[:], BIAS)
                        else:
                            nc.scalar.activation(
                                dst, ps[:],
                                mybir.ActivationFunctionType.Copy,
                                bias=BIAS, scale=1.0,
                            )
                        tcount += 1
                    # DVE fp16 2x: fold 8192 -> 512 (16-member groups)
                    a1 = a1p.tile([128, SUPER // 2], f16,
                                  name=f"a1_{sidx}_{m}", tag="a1")
                    nc.vector.tensor_max(
                        a1[:], st[:, :SUPER // 2], st[:, SUPER // 2:]
                    )
                    a2 = a2p.tile([128, SUPER // 4], f16,
                                  name=f"a2_{sidx}_{m}", tag="a2")
                    nc.vector.tensor_max(
                        a2[:], a1[:, :SUPER // 4], a1[:, SUPER // 4:]
                    )
                    a3 = a3p.tile([128, SUPER // 8], f16,
                                  name=f"a3_{sidx}_{m}", tag="a3")
                    nc.vector.tensor_max(
                        a3[:], a2[:, :SUPER // 8], a2[:, SUPER // 8:]
                    )
                    a4 = a4p.tile([128, GROUP], f16,
                                  name=f"a4_{sidx}_{m}", tag="a4")
                    nc.vector.tensor_max(
                        a4[:], a3[:, :GROUP], a3[:, GROUP:]
                    )
                    # DVE: group-winners into HIGH lanes of packed tile
                    t = sidx * MTILES + m
                    pj = pk[t % PK_BUFS]
                    hi = pj[:].bitcast(f16).rearrange(
                        "p (n two) -> p n two", two=2
                    )[:, :, 1]
                    nc.vector.tensor_copy(hi, a4[:])
                    # DVE: top-8 packed (value, group id) of the superchunk
                    nc.vector.max(
                        pacc[m][:, sidx * 8:(sidx + 1) * 8], pj[:]
                    )

            for m in range(MTILES):
                nc.sync.dma_start(packed.ap()[m * 128:(m + 1) * 128, :], pacc[m][:])

    nc.compile()
    return nc


def _get_nc():
    if "nc" not in _NC_CACHE:
        _NC_CACHE["nc"] = _build_nc()
    return _NC_CACHE["nc"]


def _make_in_maps(queries, candidates):
    import ml_dtypes

    bf = ml_dtypes.bfloat16
    q = np.asarray(queries, dtype=np.float32)
    cand = np.asarray(candidates, dtype=np.float32)
    qTh = np.ascontiguousarray(q.T.astype(bf))  # [D, B] bf16
    in_maps = []
    for i in range(N_CORES):
        cTi = np.zeros((D, PADN), dtype=bf)
        cTi[:, :SHARD] = cand[i * SHARD:(i + 1) * SHARD].T.astype(bf)
        in_maps.append({"qT": qTh, "cT": cTi})
    return in_maps


def _run_device(in_maps, trace=False):
    from concourse import bass_utils

    nc = _get_nc()
    return bass_utils.run_bass_kernel_spmd(
        nc, in_maps, core_ids=list(range(N_CORES)), trace=trace
    )


def _merge(results, queries, candidates, identifiers, num_candidates):
    K = int(num_candidates)
    q = np.asarray(queries, dtype=np.float32)
    cand = np.asarray(candidates, dtype=np.float32)
    super_base = np.repeat(np.arange(NSUPER, dtype=np.int64) * SUPER, 8)  # [SURV]
    all_u = []
    all_g = []  # [B, SURV, 16] member global ids (-1 = invalid)
    for i in range(N_CORES):
        u = np.asarray(results[i]["packed"]).view(np.uint32)       # [B, SURV]
        j = (u & 0xFFFF).astype(np.int64)                          # group id
        local = super_base[None, :, None] + j[:, :, None] \
            + GROUP * np.arange(16, dtype=np.int64)[None, None, :]  # [B,SURV,16]
        valid = local < SHARD
        u = np.where(valid.any(axis=2), u, 0)  # all-pad groups rank last
        g = np.where(valid, i * SHARD + local, -1)
        all_u.append(u)
        all_g.append(g)
    ucat = np.concatenate(all_u, axis=1)    # [B, 8*SURV]
    gcat = np.concatenate(all_g, axis=1)    # [B, 8*SURV, 16]
    # candidate set for exact rescoring: top RESCORE groups per row by rank
    nres = min(RESCORE, ucat.shape[1])
    part = np.argpartition(ucat, ucat.shape[1] - nres, axis=1)[:, -nres:]
    rows = np.arange(B)[:, None]
    gsel = gcat[rows, part].reshape(B, nres * 16)                  # [B, 16n]
    invalid = gsel < 0
    gsel_c = np.where(invalid, 0, gsel)
    # exact fp32 rescore: s[b, j] = q[b] . cand[gsel[b, j]]
    csel = cand[gsel_c]                                            # [B, 16n, D]
    vsel = np.einsum("bjd,bd->bj", csel, q, dtype=np.float32)
    vsel = np.where(invalid, -np.inf, vsel)
    # exact top-K, ties -> lower global index (matches lax.top_k)
    order = np.lexsort((gsel_c, -vsel), axis=-1)[:, :K]
    out_vals = np.take_along_axis(vsel, order, axis=1).astype(np.float32)
    out_gidx = np.take_along_axis(gsel_c, order, axis=1)
    ids = np.asarray(identifiers)
    out_ids = np.take(ids, out_gidx, axis=0)
    return out_vals, out_ids


def kernel(queries, candidates, identifiers, num_candidates):
    in_maps = _make_in_maps(queries, candidates)
    res = _run_device(in_maps, trace=False)
    return _merge(res.results, queries, candidates, identifiers, num_candidates)


# revision 8
# speedup vs baseline: 1.7104x; 1.0214x over previous
"""Brute-force KNN retrieval (B=512 queries, N=500000 candidates, D=128, top-K)
on 8 Trainium2 NeuronCores.

Strategy (sharding_hint): candidates sharded along N across the 8 cores,
queries replicated. Per core, per 8192-wide "superchunk" (4 PSUM chunks):
  - PE computes bf16 scores (fp32 PSUM), 2048 columns per PSUM tile.
  - Each PSUM chunk is drained to a quarter of a contiguous fp16(score+128)
    SBUF tile. Only ACT/DVE can read PSUM; the drain is split between ACT
    (activation Copy, ~8/9 of chunks) and DVE (tensor_scalar_add, ~1/9).
  - DVE tensor_max in fp16 (2x perf mode) folds the 8192-wide tile four
    times: 8192 -> 4096 -> 2048 -> 1024 -> 512. Element j of the result is
    the max of the 16-member group {j + 512k, k=0..15} of the superchunk.
  - DVE copies the 512 group-winners into the HIGH int16 lanes of a
    persistent packed fp32 tile whose LOW lanes hold iota (0..511); for
    positive fp16 the fp32 word orders by (fp16 value, group id).
  - DVE max8 emits the superchunk's top-8 (value, group id) in one pass.
Keep-top-8-groups-per-8192 covers the true top-100-of-500000 almost as
well as top-8 singles (a group ranks >= its members; a miss needs >=9 of
the row's top-100 inside one 8192-bin: P ~ 5e-5 per row).
The host expands surviving groups (16 members each), rescores the top
~256 groups per row exactly in fp32, and emits the exact global top-K
(ties -> lower index, like lax.top_k).
"""

import sys

for _p in ("/opt/trn_rl_repo",):
    if _p not in sys.path:
        sys.path.insert(0, _p)

import numpy as np

B, N, D = 512, 500000, 128
K = 100
N_CORES = 8
SHARD = N // N_CORES          # 62500 candidates per core
PCHUNK = 2048                 # PSUM tile width (4 banks)
SUPER = 4 * PCHUNK            # 8192-wide fold unit
GROUP = 512                   # group-winners per superchunk (16 members)
NSUPER = -(-SHARD // SUPER)   # 8
PADN = SUPER * NSUPER         # 65536 (padded shard width)
NCHUNK = PADN // PCHUNK       # 32
NSUB = PCHUNK // 512          # 4 matmuls per PSUM tile
MTILES = B // 128             # 4 query tiles
SURV = NSUPER * 8             # 64 surviving groups per (row, core)
ST_BUFS = 3                   # fp16 super drain tiles
PK_BUFS = 4                   # persistent packed tiles (iota-carrying)
RESCORE = 256                 # host rescores this many groups per row
BIAS = 128.0                  # score bias -> positive range for bit-ordering

_NC_CACHE = {}


def _is_dve_drain(t):
    # ~1/9 of chunk-tiles drain via DVE to relieve the ACT engine
    return t % 9 == 4


def _build_nc():
    import concourse.bacc as bacc
    import concourse.tile as tile
    import concourse.mybir as mybir

    f32 = mybir.dt.float32
    f16 = mybir.dt.float16
    u16 = mybir.dt.uint16
    bf16 = mybir.dt.bfloat16

    nc = bacc.Bacc(
        "TRN2", target_bir_lowering=False, debug=False, num_devices=N_CORES
    )
    qT = nc.dram_tensor("qT", [D, B], bf16, kind="ExternalInput")
    cT = nc.dram_tensor("cT", [D, PADN], bf16, kind="ExternalInput")
    packed = nc.dram_tensor("packed", [B, SURV], f32, kind="ExternalOutput")

    with tile.TileContext(nc) as tc:
        with (
            tc.tile_pool(name="q", bufs=1) as qp,
            tc.tile_pool(name="c", bufs=4) as cp,
            tc.tile_pool(name="ps", bufs=2, space="PSUM") as pp,
            tc.tile_pool(name="st", bufs=ST_BUFS) as stp,
            tc.tile_pool(name="a1", bufs=3) as a1p,
            tc.tile_pool(name="a2", bufs=3) as a2p,
            tc.tile_pool(name="a3", bufs=3) as a3p,
            tc.tile_pool(name="a4", bufs=3) as a4p,
            tc.tile_pool(name="pk", bufs=1) as pkp,
            tc.tile_pool(name="acc", bufs=1) as op,
        ):
            qt = qp.tile([128, B], bf16)
            nc.sync.dma_start(qt[:], qT.ap())

            pacc = [
                op.tile([128, SURV], f32, name=f"pacc{m}", tag=f"p{m}")
                for m in range(MTILES)
            ]
            pk = [
                pkp.tile([128, GROUP], f32, name=f"pk{j}", tag=f"k{j}")
                for j in range(PK_BUFS)
            ]
            # one-time iota into the LOW int16 lane of each packed fp32
            for j in range(PK_BUFS):
                lo = pk[j][:].bitcast(u16).rearrange(
                    "p (n two) -> p n two", two=2
                )[:, :, 0]
                nc.gpsimd.iota(lo, pattern=[[1, GROUP]], base=0,
                               channel_multiplier=0)

            tcount = 0
            for sidx in range(NSUPER):
                # DMA the 4 candidate chunks of this superchunk
                cts = []
                for q in range(4):
                    c = sidx * 4 + q
                    ct = cp.tile([128, PCHUNK], bf16, name=f"ct{c}", tag="ct")
                    nc.sync.dma_start(
                        ct[:], cT.ap()[:, c * PCHUNK:(c + 1) * PCHUNK]
                    )
                    cts.append(ct)
                for m in range(MTILES):
                    st = stp.tile([128, SUPER], f16,
                                  name=f"st{sidx}_{m}", tag="st")
                    for q in range(4):
                        ps = pp.tile([128, PCHUNK], f32,
                                     name=f"ps{sidx}_{m}_{q}", tag="ps")
                        for s in range(NSUB):
                            nc.tensor.matmul(
                                ps[:, s * 512:(s + 1) * 512],
                                qt[:, m * 128:(m + 1) * 128],
                                cts[q][:, s * 512:(s + 1) * 512],
                                start=True,
                                stop=True,
                            )
                        dst = st[:, q * PCHUNK:(q + 1) * PCHUNK]
                        if _is_dve_drain(tcount):
                            nc.vector.tensor_scalar_add(dst, ps[:], BIAS)
                        else:
                            nc.scalar.activation(
                                dst, ps[:],
                                mybir.ActivationFunctionType.Copy,
                                bias=BIAS, scale=1.0,
                            )
                        tcount += 1
                    # DVE fp16 2x: fold 8192 -> 512 (16-member groups)
                    # split tree: first fold starts after only 2 drains
                    a1 = a1p.tile([128, SUPER // 2], f16,
                                  name=f"a1_{sidx}_{m}", tag="a1")
                    nc.vector.tensor_max(
                        a1[:, :SUPER // 4],
                        st[:, :SUPER // 4], st[:, SUPER // 4:SUPER // 2]
                    )
                    nc.vector.tensor_max(
                        a1[:, SUPER // 4:],
                        st[:, SUPER // 2:3 * SUPER // 4], st[:, 3 * SUPER // 4:]
                    )
                    a2 = a2p.tile([128, SUPER // 4], f16,
                                  name=f"a2_{sidx}_{m}", tag="a2")
                    nc.vector.tensor_max(
                        a2[:], a1[:, :SUPER // 4], a1[:, SUPER // 4:]
                    )
                    a3 = a3p.tile([128, SUPER // 8], f16,
                                  name=f"a3_{sidx}_{m}", tag="a3")
                    nc.vector.tensor_max(
                        a3[:], a2[:, :SUPER // 8], a2[:, SUPER // 8:]
                    )
                    a4 = a4p.tile([128, GROUP], f16,
                                  name=f"a4_{sidx}_{m}", tag="a4")
                    nc.vector.tensor_max(
                        a4[:], a3[:, :GROUP], a3[:, GROUP:]
                    )
                    # DVE: group-winners into HIGH lanes of packed tile
                    t = sidx * MTILES + m
                    pj = pk[t % PK_BUFS]
                    hi = pj[:].bitcast(f16).rearrange(
                        "p (n two) -> p n two", two=2
                    )[:, :, 1]
                    nc.vector.tensor_copy(hi, a4[:])
                    # DVE: top-8 packed (value, group id) of the superchunk
                    nc.vector.max(
                        pacc[m][:, sidx * 8:(sidx + 1) * 8], pj[:]
                    )

            for m in range(MTILES):
                nc.sync.dma_start(packed.ap()[m * 128:(m + 1) * 128, :], pacc[m][:])

    nc.compile()
    return nc


def _get_nc():
    if "nc" not in _NC_CACHE:
        _NC_CACHE["nc"] = _build_nc()
    return _NC_CACHE["nc"]


def _make_in_maps(queries, candidates):
    import ml_dtypes

    bf = ml_dtypes.bfloat16
    q = np.asarray(queries, dtype=np.float32)
    cand = np.asarray(candidates, dtype=np.float32)
    qTh = np.ascontiguousarray(q.T.astype(bf))  # [D, B] bf16
    in_maps = []
    for i in range(N_CORES):
        cTi = np.zeros((D, PADN), dtype=bf)
        cTi[:, :SHARD] = cand[i * SHARD:(i + 1) * SHARD].T.astype(bf)
        in_maps.append({"qT": qTh, "cT": cTi})
    return in_maps


def _run_device(in_maps, trace=False):
    from concourse import bass_utils

    nc = _get_nc()
    return bass_utils.run_bass_kernel_spmd(
        nc, in_maps, core_ids=list(range(N_CORES)), trace=trace
    )


def _merge(results, queries, candidates, identifiers, num_candidates):
    K = int(num_candidates)
    q = np.asarray(queries, dtype=np.float32)
    cand = np.asarray(candidates, dtype=np.float32)
    super_base = np.repeat(np.arange(NSUPER, dtype=np.int64) * SUPER, 8)  # [SURV]
    all_u = []
    all_g = []  # [B, SURV, 16] member global ids (-1 = invalid)
    for i in range(N_CORES):
        u = np.asarray(results[i]["packed"]).view(np.uint32)       # [B, SURV]
        j = (u & 0xFFFF).astype(np.int64)                          # group id
        local = super_base[None, :, None] + j[:, :, None] \
            + GROUP * np.arange(16, dtype=np.int64)[None, None, :]  # [B,SURV,16]
        valid = local < SHARD
        u = np.where(valid.any(axis=2), u, 0)  # all-pad groups rank last
        g = np.where(valid, i * SHARD + local, -1)
        all_u.append(u)
        all_g.append(g)
    ucat = np.concatenate(all_u, axis=1)    # [B, 8*SURV]
    gcat = np.concatenate(all_g, axis=1)    # [B, 8*SURV, 16]
    # candidate set for exact rescoring: top RESCORE groups per row by rank
    nres = min(RESCORE, ucat.shape[1])
    part = np.argpartition(ucat, ucat.shape[1] - nres, axis=1)[:, -nres:]
    rows = np.arange(B)[:, None]
    gsel = gcat[rows, part].reshape(B, nres * 16)                  # [B, 16n]
    invalid = gsel < 0
    gsel_c = np.where(invalid, 0, gsel)
    # exact fp32 rescore: s[b, j] = q[b] . cand[gsel[b, j]]
    csel = cand[gsel_c]                                            # [B, 16n, D]
    vsel = np.einsum("bjd,bd->bj", csel, q, dtype=np.float32)
    vsel = np.where(invalid, -np.inf, vsel)
    # exact top-K, ties -> lower global index (matches lax.top_k)
    order = np.lexsort((gsel_c, -vsel), axis=-1)[:, :K]
    out_vals = np.take_along_axis(vsel, order, axis=1).astype(np.float32)
    out_gidx = np.take_along_axis(gsel_c, order, axis=1)
    ids = np.asarray(identifiers)
    out_ids = np.take(ids, out_gidx, axis=0)
    return out_vals, out_ids


def kernel(queries, candidates, identifiers, num_candidates):
    in_maps = _make_in_maps(queries, candidates)
    res = _run_device(in_maps, trace=False)
    return _merge(res.results, queries, candidates, identifiers, num_candidates)
